# revision 4
# baseline (speedup 1.0000x reference)
"""Trainium2 Bass kernel for nn_Conv2dP4P4 (P4->P4 group-equivariant conv).

Math (verified vs reference):
  W2 = w.reshape(64,4,64,3,3).sum(1)                  # tap-sum absorbs the
                                                      # reference's group-sum
  out[b, 16q+m', i] = rot90( conv_valid(x[b,:,j], rot90(W2[16q:16q+16], k=i)),
                             k=-i )   with j = (q+i)%4

Per core (8 cores, batch-sharded: 2 batches/core), per unit (b, j):
  - slab S = x[b,:,j] in SBUF [64c, 128*128] (fp32r)
  - 63 psum tiles [64, 2*130], each = 2 output rows; 9 accumulated fp32r
    matmuls (taps), kw realized as even psum column offsets (s_off trick),
    kh as rhs row offsets. M=64 packs all 4 rotations:
    m-hat blocks: [0:16]=i0, [16:32]=i2, [32:48]=i1, [48:64]=i3.
  - i0/i2 evicted m-on-partitions (i2 fully reversed read); i1/i3 go
    through PE transposes (stacked [128,128]) to get w-on-partitions, i3
    from a reversed psum window; all output rotations become pure
    addressing.
"""
import sys
import numpy as np

sys.path.insert(0, "/opt/trn_rl_repo")

B, C, G, H, W = 16, 64, 4, 128, 128
OUT = 64
HO = H - 2  # 126
NCORES = 8
BPC = B // NCORES  # batches per core

_cache = {}


def _build_weights(w: np.ndarray) -> np.ndarray:
    """w: [256, 64, 3, 3] -> LH [64c, 4j * 9t * 64m] fp32 (c partition-major)."""
    W2 = w.reshape(OUT, 4, C, 3, 3).sum(axis=1)  # [64, 64, 3, 3]
    # block order: pos0=i0, pos1=i2, pos2=i1, pos3=i3
    iorder = [0, 2, 1, 3]
    LH = np.zeros((4, 9, C, 64), dtype=np.float32)  # [j, t, c, mhat]
    for j in range(4):
        for pos, i in enumerate(iorder):
            q = (j - i) % 4
            Ki = np.rot90(W2[16 * q:16 * (q + 1)], k=i, axes=(-2, -1))  # [16,64,3,3]
            for kh in range(3):
                for kw in range(3):
                    t = kh * 3 + kw
                    # LH[j,t][c, pos*16+m'] = Ki[m', c, kh, kw]
                    LH[j, t, :, pos * 16:(pos + 1) * 16] = Ki[:, :, kh, kw].T
    return np.ascontiguousarray(LH.transpose(2, 0, 1, 3).reshape(C, 4 * 9 * 64))


def _build_program():
    import concourse.bass as bass
    import concourse.tile as tile
    from concourse import bacc, mybir

    nc = bacc.Bacc("TRN2", target_bir_lowering=False, debug=False)
    x_in = nc.dram_tensor("x_in", [BPC, C, G, H, W], mybir.dt.float32r,
                          kind="ExternalInput").ap()
    w_in = nc.dram_tensor("w_in", [C, 4 * 9 * 64], mybir.dt.float32r,
                          kind="ExternalInput").ap()
    eye_in = nc.dram_tensor("eye_in", [128, 128], mybir.dt.float32,
                            kind="ExternalInput").ap()
    o_out = nc.dram_tensor("o_out", [BPC, OUT, 4, HO, HO], mybir.dt.float32,
                           kind="ExternalOutput").ap()

    f32 = mybir.dt.float32
    f32r = mybir.dt.float32r
    CH = 32  # stage chunk rows

    with tile.TileContext(nc, trace_sim=False) as tc:
        with tc.tile_pool(name="wtp", bufs=1) as wtp, \
             tc.tile_pool(name="slab", bufs=1) as slabp, \
             tc.tile_pool(name="st0", bufs=2) as st0p, \
             tc.tile_pool(name="st2", bufs=2) as st2p, \
             tc.tile_pool(name="stT", bufs=2) as stTp, \
             tc.tile_pool(name="scr", bufs=2) as scrp, \
             tc.tile_pool(name="psc", bufs=4, space="PSUM") as pscp, \
             tc.tile_pool(name="pst", bufs=2, space="PSUM") as pstp:

            wt = wtp.tile([C, 4 * 9 * 64], f32r)
            nc.sync.dma_start(wt[:], w_in)
            eye = wtp.tile([128, 128], f32)
            nc.sync.dma_start(eye[:], eye_in)

            for u in range(2 * 4):
                b, j = u // 4, u % 4
                S = slabp.tile([C, H * W + 8], f32r, tag="slab")
                src = x_in[b, :, j].rearrange("c h w -> c (h w)")
                # two halves so the 2nd overlaps compute on the 1st
                nc.sync.dma_start(S[:, 0:H * W // 2], src[:, 0:H * W // 2])
                nc.sync.dma_start(S[:, H * W // 2:H * W], src[:, H * W // 2:])

                # output stages
                stT1 = stTp.tile([126, 16 * HO], f32, tag="t1")
                stT3 = stTp.tile([126, 16 * HO], f32, tag="t3")

                st0_tiles = {}
                st2_tiles = {}
                psum2 = None

                q0, q2 = j, (j - 2) % 4
                q1, q3 = (j - 1) % 4, (j - 3) % 4

                for k in range(63):
                    g, half = k // 2, k % 2
                    pt = pscp.tile([64, 2 * 130], f32, tag="conv")
                    pt3 = pt[:].rearrange("m (r x) -> m r x", r=2)
                    # --- 9 tap matmuls, accumulate in psum
                    for t in range(9):
                        kh, kw = t // 3, t % 3
                        s_off = kw % 2
                        p_off = 2 + s_off - kw
                        base = (2 * k + kh) * W + s_off
                        rhs = S[:, base:base + 256]
                        lhsT = wt[:, (j * 9 + t) * 64:(j * 9 + t) * 64 + 64]
                        nc.tensor.matmul(
                            pt3[:, :, p_off:p_off + 128], lhsT, rhs,
                            start=(t == 0), stop=(t == 8),
                            skip_group_check=True)

                    # --- i0: normal 32-lane copy (valid half [0:16])
                    c0 = (2 * k) // CH
                    if c0 not in st0_tiles:
                        st0_tiles[c0] = st0p.tile([32, CH * HO], f32, tag="st0", name="st0c")
                    row_in_chunk = 2 * k - c0 * CH
                    nc.vector.tensor_copy(
                        st0_tiles[c0][:, row_in_chunk * HO:(row_in_chunk + 2) * HO],
                        pt3[0:32, :, 2:128])
                    if 2 * k + 2 >= min(c0 * CH + CH, HO):  # chunk complete
                        rows = min(CH, HO - c0 * CH)
                        nc.sync.dma_start(
                            o_out[b, 16 * q0:16 * (q0 + 1), 0,
                                  c0 * CH:c0 * CH + rows]
                            .rearrange("m u v -> m (u v)"),
                            st0_tiles[c0][0:16, 0:rows * HO])

                    # --- i2: fully reversed 32-lane copy (valid half [16:32])
                    slot = 124 - 2 * k  # first of the 2 slots (ascending)
                    c2 = slot // CH
                    if c2 not in st2_tiles:
                        st2_tiles[c2] = st2p.tile([32, CH * HO], f32, tag="st2", name="st2c")
                    sl = slot - c2 * CH
                    nc.vector.tensor_copy(
                        st2_tiles[c2][:, sl * HO:(sl + 2) * HO],
                        pt3[0:32, ::-1, 127:1:-1])
                    if slot == c2 * CH:  # chunk complete (fills downward)
                        rows = min(CH, HO - c2 * CH)
                        nc.sync.dma_start(
                            o_out[b, 16 * q2:16 * (q2 + 1), 2,
                                  c2 * CH:c2 * CH + rows]
                            .rearrange("m u v -> m (u v)"),
                            st2_tiles[c2][16:32, 0:rows * HO])

                    # --- T-path fills for this tile's two row-blocks
                    if half == 0:
                        scrN = scrp.tile([128, 128], f32, tag="scrN")
                        scrR = scrp.tile([128, 128], f32, tag="scrR")
                    for r in range(2):
                        s = 2 * half + r
                        nc.vector.tensor_copy(
                            scrN[32 * s:32 * s + 32, :],
                            pt3[32:64, r, 2:130])
                        nc.vector.tensor_copy(
                            scrR[32 * s:32 * s + 32, :],
                            pt3[32:64, r, 127::-1])

                    # --- after each group of 2 tiles (4 rows): transpose+stage
                    last = (k == 62)
                    if half == 1 or last:
                        nstk = 2 if (last and half == 0) else 4
                        psum2 = pstp.tile([128, 128], f32, tag="p2")
                        psum3 = pstp.tile([128, 128], f32, tag="p3")
                        nc.tensor.transpose(psum2[:, 0:32 * nstk],
                                            scrN[0:32 * nstk, :],
                                            eye[0:32 * nstk, 0:32 * nstk])
                        nc.tensor.transpose(psum3[:, 0:32 * nstk],
                                            scrR[0:32 * nstk, :],
                                            eye[0:32 * nstk, 0:32 * nstk])
                        p2r = psum2[0:126, 0:32 * nstk].rearrange(
                            "p (s mm) -> p s mm", s=nstk)
                        p3r = psum3[0:126, 0:32 * nstk].rearrange(
                            "p (s mm) -> p s mm", s=nstk)
                        t1r = stT1[:].rearrange("p (mm v) -> p mm v", v=HO)
                        t3r = stT3[:].rearrange("p (mm v) -> p mm v", v=HO)
                        r0 = 4 * g  # first abs row in group
                        # i1: v-slot = 125 - r_abs (descending in s)
                        nc.vector.tensor_copy(
                            t1r[:, :, 125 - r0 - nstk + 1:126 - r0][:, :, ::-1],
                            p2r[:, 0:nstk, 0:16].rearrange("p s mm -> p mm s"))
                        # i3: v-slot = r_abs (ascending)
                        nc.vector.tensor_copy(
                            t3r[:, :, r0:r0 + nstk],
                            p3r[:, 0:nstk, 16:32].rearrange("p s mm -> p mm s"))

                # --- flush stages to HBM (chunks were flushed in-loop)
                nc.sync.dma_start(
                    o_out[b, 16 * q1:16 * (q1 + 1), 1].rearrange("m u v -> u m v"),
                    stT1[:].rearrange("p (mm v) -> p mm v", v=HO))
                nc.sync.dma_start(
                    o_out[b, 16 * q3:16 * (q3 + 1), 3].rearrange("m u v -> u m v"),
                    stT3[:].rearrange("p (mm v) -> p mm v", v=HO))

    nc.compile()
    return nc


def kernel(x: np.ndarray, w: np.ndarray) -> np.ndarray:
    from concourse.bass_utils import run_bass_kernel_spmd

    if "nc" not in _cache:
        _cache["nc"] = _build_program()
    nc = _cache["nc"]

    wlh = _build_weights(np.asarray(w, dtype=np.float32))
    x = np.ascontiguousarray(np.asarray(x, dtype=np.float32))
    eye = np.eye(128, dtype=np.float32)
    in_maps = [{"x_in": x[c * BPC:(c + 1) * BPC], "w_in": wlh, "eye_in": eye}
               for c in range(NCORES)]
    res = run_bass_kernel_spmd(nc, in_maps, list(range(NCORES)))
    out = np.concatenate([res.results[c]["o_out"] for c in range(NCORES)], axis=0)
    return out.astype(np.float32)
